# revision 1
# baseline (speedup 1.0000x reference)
"""Trainium2 Bass kernel for the AnaphoricityScorer (coref pair scorer).

Reference computation (per mention row m, antecedent slot j):
    b = all_mentions[idx[m, j]]                       # gather, 1024-dim
    pair = [a_m, b, a_m * b, pw[m, j]]                # 3136-dim
    h = leaky_relu(pair @ W1 + b1)                    # 1024-dim
    score[m, j] = rough[m, j] + h @ W_out + b_out
    out[m] = [EPS, score[m, 0..49]]                   # [1024, 51]

Strategy (8 NeuronCores, data-parallel over the 1024-mention batch):
  * 128 mentions per core; all_mentions + FFNN weights replicated.
  * Decompose W1 by pair-feature block: W1 = [W1_a; W1_b; W1_ab; W1_pw].
      - a-term:  A1^T = (W1_a^T a^T) computed once per core (tiny).
      - b-term:  recomputed per pair as fp8-DoubleRow matmuls over the
        already-gathered bT (an earlier BM-precompute+AllGather+gather
        design was SLOWER: the gather stream is descriptor-rate-bound on
        the gpsimd engine, so +13.4 GFLOP of cheap fp8 PE beats 13MB of
        extra gathers + a 16MB collective).
      - ab-term: irreducible 13.4 GFLOP/core, also fp8 DoubleRow: abT and
        W1ab quantized to e4m3; each matmul contracts two 128-row e-chunks
        via 3-D APs [128, 2, cols].
        (total rel err ~1.6e-2 vs 1.6e-3 all-bf16; gate is 2e-2.)
      - pw-term: K=64 bf16 matmul.
  * dma_gather(transpose=True) both gathers antecedent rows AND delivers them
    feature-major ([emb%128, emb//128, paircol]) - exactly the moving-operand
    layout the matmul needs. Pair columns are ordered g = j*128 + m so each
    128-column group shares one antecedent slot j across all 128 mentions.
    Gathers stay bf16: the transpose works at 16-bit granularity (fp8
    rows come back pair-interleaved), and the gather stream is
    descriptor-rate limited, so fp8 would not be faster anyway.
  * Per 512-column block: h^T psum accumulates ab-term (4 DoubleRow MM) +
    pw-term on the TensorEngine; BM^T + A1^T are pre-combined on the
    VectorEngine and added straight into PSUM; leaky-relu on the
    ScalarEngine -> bf16 h^T; layer 2 is N=1 matmuls (lhsT = h^T j-slices,
    rhs = W_out chunk, auto-FWL) producing ffnn in [mention, j] layout.
  * Overlap: bT gathers prefetched 6 blocks ahead on the single SWDGE
    queue (multi-queue gathers measured slower once made stable - the
    gpsimd descriptor generation is the serial resource).
  * DBG_REPEAT wraps the whole body (prologue + AllGather + main loop) for
    device-time measurement via repetition deltas; production is REPEAT=1.
"""

import numpy as np
import ml_dtypes

CORES = 8
BATCH = 1024
B_LOC = BATCH // CORES          # 128 mentions per core
N_ANTS = 50
EMB = 1024
PW = 64
HID = 1024
N_MENT = 8192
G = B_LOC * N_ANTS              # 6400 pair columns per core
EPS = 1e-7

COLS_PER_BLK = 512              # pair columns per block (4 antecedent slots)
N_BLK = (G + COLS_PER_BLK - 1) // COLS_PER_BLK   # 13 (12 full + 1 half)

SIM_SAFE = False                # set True before get_nc() for CoreSim runs
import os as _os
DBG_NO_COLL = _os.environ.get("KDBG_NO_COLL", "0") == "1"   # skip AllGather (wrong results)
DBG_NBLK = int(_os.environ.get("KDBG_NBLK", "0"))           # limit main-loop blocks
DBG_NO_LRELU = _os.environ.get("KDBG_NO_LRELU", "0") == "1"
DBG_REPEAT = int(_os.environ.get("KDBG_REPEAT", "1"))  # repeat main loop (timing)
DBG_GONLY = _os.environ.get("KDBG_GONLY", "0") == "1"  # gathers only (wrong results)

_BF16 = ml_dtypes.bfloat16

_cache = {}


def _build_nc():
    from contextlib import ExitStack

    import concourse.bacc as bacc
    import concourse.mybir as mybir
    import concourse.tile as tile

    f32 = mybir.dt.float32
    bf16 = mybir.dt.bfloat16
    f8 = mybir.dt.float8e4
    i16 = mybir.dt.int16
    AF = mybir.ActivationFunctionType
    ALU = mybir.AluOpType
    DR = mybir.MatmulPerfMode.DoubleRow

    nc = bacc.Bacc("TRN2", target_bir_lowering=False, debug=False,
                   num_devices=CORES)

    def inp(name, shape, dt):
        return nc.dram_tensor(name, shape, dt, kind="ExternalInput")

    am_d = inp("am", [N_MENT, EMB], bf16)         # all_mentions (bT gathers)
    aT_d = inp("aT", [128, 8, B_LOC], bf16)       # a^T  [e%128, e//128, m]
    aTr_d = inp("aTr", [128, 8, 512], bf16)       # a^T tiled x4 along cols
    pwT_d = inp("pwT", [PW, N_ANTS, B_LOC], bf16)  # pw^T [f, j, m]
    rough_d = inp("rough", [B_LOC, N_ANTS], f32)
    w1a_d = inp("w1a", [128, 8, HID], bf16)       # W1 rows 0:1024    [e%128, e//128, h]
    w1b_d = inp("w1b", [128, 8, HID], f8)         # W1 rows 1024:2048, fp8
    w1ab_d = inp("w1ab", [128, 8, HID], f8)       # W1 rows 2048:3072, fp8
    w1pw_d = inp("w1pw", [PW, HID], bf16)         # W1 rows 3072:3136
    wout_d = inp("wout", [128, 8], bf16)          # W_out [h%128, h//128]
    b1T_d = inp("b1T", [128, 8], f32)             # b1    [h%128, h//128]
    bout_d = inp("boutr", [128, 1], f32)          # b_out replicated per partition
    idx_d = inp("idx", [128, G // 16], i16)       # wrapped gather indices
    out_d = nc.dram_tensor("out", [B_LOC, 1 + N_ANTS], f32, kind="ExternalOutput")

    with tile.TileContext(nc) as tc, ExitStack() as ctx:
        const = ctx.enter_context(tc.tile_pool(name="const", bufs=1))
        dram = ctx.enter_context(tc.tile_pool(name="dram", bufs=1, space="DRAM"))
        gpool = ctx.enter_context(tc.tile_pool(name="gather", bufs=3))
        wpool = ctx.enter_context(tc.tile_pool(name="work", bufs=3))
        hpool = ctx.enter_context(tc.tile_pool(name="hsb", bufs=3))
        plp = ctx.enter_context(tc.tile_pool(name="prolog", bufs=1))
        pp = ctx.enter_context(tc.tile_pool(name="psum", bufs=6, space="PSUM"))
        fp = ctx.enter_context(tc.tile_pool(name="fpsum", bufs=2, space="PSUM"))

        def load(dram_t, shape, dt, eng=None):
            t = const.tile(shape, dt, tag=dram_t.name)
            (eng or nc.sync).dma_start(t[:], dram_t[:])
            return t

        # load order matters: idx tiles + small consts first so block-0
        # gathers and matmuls can start while the big weights stream in.
        # Big weights go on the scalar HWDGE queue, small/critical on sync.
        idx_sb = load(idx_d, [128, G // 16], i16)
        w1b8 = load(w1b_d, [128, 8, HID], f8)
        aT = load(aT_d, [128, 8, B_LOC], bf16, eng=nc.scalar)
        aTr = load(aTr_d, [128, 8, 512], bf16, eng=nc.scalar)
        wout = load(wout_d, [128, 8], bf16, eng=nc.scalar)
        b1T = load(b1T_d, [128, 8], f32, eng=nc.scalar)
        bout = load(bout_d, [128, 1], f32, eng=nc.scalar)
        rough = load(rough_d, [B_LOC, N_ANTS], f32, eng=nc.scalar)
        w1ab = load(w1ab_d, [128, 8, HID], f8, eng=nc.scalar)
        pwT = load(pwT_d, [PW, N_ANTS, B_LOC], bf16, eng=nc.scalar)
        w1pw = load(w1pw_d, [PW, HID], bf16, eng=nc.scalar)
        w1a = plp.tile([128, 8, HID], bf16, tag="w1a")
        nc.scalar.dma_start(w1a[:], w1a_d[:])

        a1Tr = const.tile([128, 8, 512], bf16, tag="a1Tr")
        scores = const.tile([B_LOC, 1 + N_ANTS], f32, tag="scores")
        nc.vector.memset(scores[:, 0:1], EPS)

        nblk = N_BLK if DBG_NBLK == 0 else max(DBG_NBLK, 0)
        PREF = 6   # bT gathers issued this many blocks ahead

        def block_geom(B):
            nj = min(4, N_ANTS - B * 4)
            return nj, nj * 128, B * (COLS_PER_BLK // 16)

        def issue_bT(B):
            nj, NCOL, i0 = block_geom(B)
            t = gpool.tile([128, 8, NCOL], bf16, tag="bT", name=f"bT{B}", bufs=6)
            nc.gpsimd.dma_gather(t[:], am_d.ap(),
                                 idx_sb[:, i0:i0 + NCOL // 16],
                                 NCOL, NCOL, EMB, transpose=True)
            return t

        def body(rep):
            bT_q = {}
            for i in range(min(PREF, nblk)):
                bT_q[i] = issue_bT(i)

            # A1^T = (a @ W1_a + b1)^T : [h%128, h//128, m], bf16,
            # replicated x4 along cols so the per-block add spans NCOL.
            for ch in range(8):
                ps = pp.tile([128, B_LOC], f32, tag="hp")
                for ce in range(8):
                    nc.tensor.matmul(ps[:], w1a[:, ce, ch * 128:(ch + 1) * 128],
                                     aT[:, ce, :], start=(ce == 0), stop=(ce == 7))
                for j in range(4):
                    nc.vector.tensor_scalar_add(a1Tr[:, ch, j * 128:(j + 1) * 128],
                                                ps[:], b1T[:, ch:ch + 1])

            # ---- main loop over pair-column blocks
            for bi in range(nblk):
                nj, NCOL, i0 = block_geom(bi)
                j0 = bi * 4

                bT = bT_q.pop(bi)
                if bi + PREF < nblk:
                    bT_q[bi + PREF] = issue_bT(bi + PREF)

                if DBG_GONLY:   # gather throughput probe: skip all compute
                    continue

                abT = wpool.tile([128, 8, NCOL], f8, tag="abT")
                nc.vector.tensor_mul(abT[:], bT[:], aTr[:, :, 0:NCOL])
                b8 = wpool.tile([128, 8, NCOL], f8, tag="b8")
                nc.vector.tensor_scalar_mul(b8[:], bT[:], 1.0)

                ffps = fp.tile([128, nj], f32, tag="ff")
                for ch in range(8):
                    hp = pp.tile([128, NCOL], f32, tag="hp")
                    # fp8 DoubleRow: each matmul contracts two 128-row
                    # e-chunks (lhsT/rhs 3-D APs [128, 2, cols])
                    for ce in range(4):
                        nc.tensor.matmul(hp[:],
                                         w1ab[:, 2 * ce:2 * ce + 2,
                                              ch * 128:(ch + 1) * 128],
                                         abT[:, 2 * ce:2 * ce + 2, :],
                                         start=(ce == 0), stop=False,
                                         perf_mode=DR)
                    # b-term recomputed per pair in fp8 DoubleRow (cheaper
                    # than BM gather+AllGather: gathers are descriptor-bound)
                    for ce in range(4):
                        nc.tensor.matmul(hp[:],
                                         w1b8[:, 2 * ce:2 * ce + 2,
                                              ch * 128:(ch + 1) * 128],
                                         b8[:, 2 * ce:2 * ce + 2, :],
                                         start=False, stop=False,
                                         perf_mode=DR)
                    nc.tensor.matmul(hp[:], w1pw[:, ch * 128:(ch + 1) * 128],
                                     pwT[:, j0:j0 + nj, :], start=False,
                                     stop=True)
                    # A1^T added straight into PSUM on DVE
                    nc.vector.tensor_add(hp[:], hp[:], a1Tr[:, ch, 0:NCOL])

                    hT = hpool.tile([128, NCOL], bf16, tag="hT")
                    # leaky_relu(x) = max(x, 0.01*x)
                    if SIM_SAFE or DBG_NO_LRELU:  # CoreSim lacks Lrelu
                        lt = hpool.tile([128, NCOL], f32, tag="lt")
                        nc.vector.tensor_scalar_mul(lt[:], hp[:], 0.01)
                        nc.vector.tensor_max(hT[:], hp[:], lt[:])
                    else:
                        nc.scalar.activation(hT[:], hp[:], AF.Lrelu, alpha=0.01)
                    for j in range(nj):
                        nc.tensor.matmul(ffps[:, j:j + 1],
                                         hT[:, j * 128:(j + 1) * 128],
                                         wout[:, ch:ch + 1],
                                         start=(ch == 0 and j == 0),
                                         stop=(ch == 7 and j == nj - 1))

                # scores[:, 1+j0 : 1+j0+nj] = ffnn + b_out + rough
                nc.vector.scalar_tensor_tensor(scores[:, 1 + j0:1 + j0 + nj],
                                               ffps[:], bout[:, 0:1],
                                               rough[:, j0:j0 + nj],
                                               op0=ALU.add, op1=ALU.add)

        for rep in range(max(DBG_REPEAT, 1)):
            body(rep)

        nc.sync.dma_start(out_d[:], scores[:])

    nc.compile()
    return nc


def _shard(inputs):
    am = np.asarray(inputs["all_mentions"], np.float32)
    mb_ = np.asarray(inputs["mentions_batch"], np.float32)
    pw = np.asarray(inputs["pw_batch"], np.float32)
    rough = np.asarray(inputs["top_rough_scores_batch"], np.float32)
    W1 = np.asarray(inputs["W1"], np.float32)
    b1 = np.asarray(inputs["b1"], np.float32)
    Wout = np.asarray(inputs["W_out"], np.float32)
    bout = np.asarray(inputs["b_out"], np.float32)
    idx = np.asarray(inputs["top_indices_batch"])

    am_bf = am.astype(_BF16)

    def wblock(rows, dt=_BF16):  # [1024, 1024] -> [128, 8, 1024] (e%128, e//128, h)
        return np.ascontiguousarray(
            rows.reshape(8, 128, HID).transpose(1, 0, 2)).astype(dt)

    w1a = wblock(W1[0:1024])
    w1b = wblock(W1[1024:2048], ml_dtypes.float8_e4m3)
    w1ab = wblock(W1[2048:3072], ml_dtypes.float8_e4m3)
    w1pw = np.ascontiguousarray(W1[3072:3136]).astype(_BF16)          # [64, 1024]
    wout = np.ascontiguousarray(Wout[:, 0].reshape(8, 128).T).astype(_BF16)
    b1T = np.ascontiguousarray(b1.reshape(8, 128).T).astype(np.float32)
    boutr = np.full((128, 1), bout[0], np.float32)

    def wrap_idx(flat, pad_cols):
        # [16, n/16] wrapped block, replicated across all 8 GPSIMD-core
        # partition groups (the odd Q7 core reads partitions 16-31).
        return np.tile(flat.reshape(pad_cols, 16).T, (8, 1))

    in_maps = []
    for c in range(CORES):
        sl = slice(c * B_LOC, (c + 1) * B_LOC)
        aT = np.ascontiguousarray(
            mb_[sl].T.reshape(8, 128, B_LOC).transpose(1, 0, 2)).astype(_BF16)
        aTr = np.ascontiguousarray(np.tile(aT, (1, 1, 4)))
        pwT = np.ascontiguousarray(pw[sl].transpose(2, 1, 0)).astype(_BF16)
        idx_flat = np.ascontiguousarray(idx[sl].astype(np.int16).T).reshape(G)
        in_maps.append({
            "am": am_bf,
            "aT": aT,
            "aTr": aTr,
            "pwT": pwT,
            "rough": np.ascontiguousarray(rough[sl]),
            "w1a": w1a, "w1b": w1b, "w1ab": w1ab, "w1pw": w1pw,
            "wout": wout, "b1T": b1T, "boutr": boutr,
            "idx": wrap_idx(idx_flat, G // 16),
        })
    return in_maps


def get_nc():
    if "nc" not in _cache:
        _cache["nc"] = _build_nc()
    return _cache["nc"]


def kernel(**inputs):
    from concourse.bass_utils import run_bass_kernel_spmd

    nc = get_nc()
    in_maps = _shard(inputs)
    res = run_bass_kernel_spmd(nc, in_maps, core_ids=list(range(CORES)))
    outs = [r["out"] for r in res.results]
    return np.concatenate(outs, axis=0)



# revision 9
# speedup vs baseline: 16.7712x; 16.7712x over previous
"""Trainium2 Bass kernel for the AnaphoricityScorer (coref pair scorer). v5

Reference computation (per mention row m, antecedent slot j):
    b = all_mentions[idx[m, j]]                       # gather, 1024-dim
    pair = [a_m, b, a_m * b, pw[m, j]]                # 3136-dim
    h = leaky_relu(pair @ W1 + b1)                    # 1024-dim
    score[m, j] = rough[m, j] + h @ W_out + b_out
    out[m] = [EPS, score[m, 0..49]]                   # [1024, 51]

Strategy (8 NeuronCores, data-parallel over the 1024-mention batch).

Measured HW model (repetition deltas + engine-isolation probes): the PE
streams ONE moving column per cycle per pass regardless of dtype (the
CoreSim cost model's 0.5 cyc/row for fp8-DR is NOT what HW does); fp8
DoubleRow contracts 256 rows/pass (2 planes), bf16 128 rows/pass, plus
~45ns of per-instruction overhead.  The kernel is PE-bound (probe: the
PE path alone is 268us of a 276us body; gathers 52us, DVE ~12us/blk,
ACT ~8us/blk all hide behind it), so the design minimizes column-passes:
  * per 512-column block and 128-hidden chunk (x8):
      - ab-term: 4 fp8-DR passes (abT = bT*aTr on DVE -> e4m3, 8 planes)
      - b-term:  4 fp8-DR passes (bT -> e4m3 cast split ACT/DVE)
      - pw-term: 1 fp8-DR pass (pw^T e4m3 + a zero plane, DMA'd per
        block from a host-packed tensor)
      -> 9 passes = the plane floor ceil(17/2); the a-term does NOT ride
        the PE: A1^T = (a @ W1_a + b1)^T is computed once per core,
        replicated x4 along columns, and added into each h^T PSUM tile
        on the DVE (full bf16 precision, and the DVE has slack).
  * layer 2 merged: ONE matmul per chunk (W_out chunk stationary,
    h^T moving) accumulating into a [1, 512] PSUM row; scores leave
    flat as out[1, 6400] (g = j*128 + m) and the host reassembles
    [1024, 51] + EPS column (the per-j N=1 matmuls of earlier versions
    cost ~4us/block in stationary reloads).
  * software pipelining: per iteration, DVE/ACT/DMA prepare block N's
    moving tensors while the PE runs block N-1's matmuls and block
    N-2's layer-2; no engine's in-order queue ever waits on a
    same-block cross-engine round trip.
  * dma_gather(transpose=True) delivers gathered rows feature-major;
    gathers measured ~52us/body, far from critical.  num_idxs per
    gather stays at 512 (1024-idx gathers fault the device).
  * DBG_REPEAT wraps the whole body for device-time measurement via
    repetition deltas; production is REPEAT=1.
"""

import numpy as np
import ml_dtypes

CORES = 8
BATCH = 1024
B_LOC = BATCH // CORES          # 128 mentions per core
N_ANTS = 50
EMB = 1024
PW = 64
HID = 1024
N_MENT = 8192
G = B_LOC * N_ANTS              # 6400 pair columns per core
EPS = 1e-7

COLS_PER_BLK = 512              # pair columns per block (4 antecedent slots)
N_BLK = (G + COLS_PER_BLK - 1) // COLS_PER_BLK   # 13 (12 full + 1 half)

SIM_SAFE = False                # set True before get_nc() for CoreSim runs
import os as _os
DBG_NBLK = int(_os.environ.get("KDBG_NBLK", "0"))           # limit main-loop blocks
DBG_NO_LRELU = _os.environ.get("KDBG_NO_LRELU", "0") == "1"
DBG_REPEAT = int(_os.environ.get("KDBG_REPEAT", "1"))  # repeat main loop (timing)
DBG_GONLY = _os.environ.get("KDBG_GONLY", "0") == "1"  # gathers only (wrong results)
DBG_PROBE = _os.environ.get("KDBG_PROBE", "")  # "pe": skip DVE/ACT feeds; "ve": skip matmuls

_BF16 = ml_dtypes.bfloat16
_F8 = ml_dtypes.float8_e4m3

_cache = {}


def _build_nc():
    from contextlib import ExitStack

    import concourse.bacc as bacc
    import concourse.mybir as mybir
    import concourse.tile as tile

    f32 = mybir.dt.float32
    bf16 = mybir.dt.bfloat16
    f8 = mybir.dt.float8e4
    i16 = mybir.dt.int16
    AF = mybir.ActivationFunctionType
    ALU = mybir.AluOpType
    DR = mybir.MatmulPerfMode.DoubleRow

    probe_pe = DBG_PROBE == "pe"
    probe_ve = DBG_PROBE == "ve"

    nc = bacc.Bacc("TRN2", target_bir_lowering=False, debug=False,
                   num_devices=CORES)

    def inp(name, shape, dt):
        return nc.dram_tensor(name, shape, dt, kind="ExternalInput")

    am_d = inp("am", [N_MENT, EMB], bf16)         # all_mentions (bT gathers)
    aT_d = inp("aT", [128, 8, B_LOC], bf16)       # a^T  [e%128, e//128, m]
    aTr_d = inp("aTr", [128, 8, 512], bf16)       # a^T tiled x4 along cols
    px_d = inp("px", [128, 2, N_ANTS, B_LOC], f8)  # planes: pwT8, 0
    roughf_d = inp("roughf", [1, G], f32)         # rough, flat g = j*128+m
    w1a_d = inp("w1a", [128, 8, HID], bf16)       # W1 rows 0:1024    [e%128, e//128, h]
    w1bx_d = inp("w1bx", [128, 10, HID], f8)      # [W1_b(8); W1_pw pad; 0]
    w1ab_d = inp("w1ab", [128, 8, HID], f8)       # W1 rows 2048:3072, fp8
    wout_d = inp("wout", [128, 8], bf16)          # W_out [h%128, h//128]
    b1x_d = inp("b1x", [128, 8, B_LOC], bf16)     # b1 [h%128, h//128] bcast x128 m
    bout_d = inp("boutr", [128, 1], f32)          # b_out replicated per partition
    idx_d = inp("idx", [128, G // 16], i16)       # wrapped gather indices
    out_d = nc.dram_tensor("out", [1, G], f32, kind="ExternalOutput")

    with tile.TileContext(nc) as tc, ExitStack() as ctx:
        const = ctx.enter_context(tc.tile_pool(name="const", bufs=1))
        gpool = ctx.enter_context(tc.tile_pool(name="gather", bufs=3))
        wpool = ctx.enter_context(tc.tile_pool(name="work", bufs=3))
        bpool = ctx.enter_context(tc.tile_pool(name="bstr", bufs=3))
        hpool = ctx.enter_context(tc.tile_pool(name="hsb", bufs=18))
        spool = ctx.enter_context(tc.tile_pool(name="scout", bufs=2))
        plp = ctx.enter_context(tc.tile_pool(name="prolog", bufs=1))
        pp = ctx.enter_context(tc.tile_pool(name="psum", bufs=4, space="PSUM"))
        fp = ctx.enter_context(tc.tile_pool(name="fpsum", bufs=2, space="PSUM"))
        plps = ctx.enter_context(tc.tile_pool(name="plpsum", bufs=1, space="PSUM"))

        def load(dram_t, shape, dt, eng=None):
            t = const.tile(shape, dt, tag=dram_t.name)
            (eng or nc.sync).dma_start(t[:], dram_t[:])
            return t

        # load order matters: idx tiles + small consts first so block-0
        # gathers and matmuls can start while the big weights stream in.
        idx_sb = load(idx_d, [128, G // 16], i16)
        w1bx = load(w1bx_d, [128, 10, HID], f8)
        aT = load(aT_d, [128, 8, B_LOC], bf16, eng=nc.scalar)
        aTr = load(aTr_d, [128, 8, 512], bf16, eng=nc.scalar)
        wout = load(wout_d, [128, 8], bf16, eng=nc.scalar)
        b1x = load(b1x_d, [128, 8, B_LOC], bf16, eng=nc.scalar)
        bout = load(bout_d, [128, 1], f32, eng=nc.scalar)
        roughf = load(roughf_d, [1, G], f32, eng=nc.scalar)
        w1ab = load(w1ab_d, [128, 8, HID], f8, eng=nc.scalar)
        w1a = plp.tile([128, 8, HID], bf16, tag="w1a")
        nc.scalar.dma_start(w1a[:], w1a_d[:])

        a1Tr = const.tile([128, 8, 512], bf16, tag="a1Tr")
        dadd = const.tile([128, 512], f32, tag="dadd")   # probe_ve add sink
        if probe_ve:
            nc.vector.memset(dadd[:], 0.0)

        nblk = N_BLK if DBG_NBLK == 0 else max(DBG_NBLK, 0)
        PREF = 3   # bT gathers issued this many blocks ahead

        def block_geom(B):
            nj = min(4, N_ANTS - B * 4)
            return nj, nj * 128, B * (COLS_PER_BLK // 16)

        def issue_bT(B):
            nj, NCOL, i0 = block_geom(B)
            t = gpool.tile([128, 8, NCOL], bf16, tag="bT", name=f"bT{B}", bufs=4)
            nc.gpsimd.dma_gather(t[:], am_d.ap(),
                                 idx_sb[:, i0:i0 + NCOL // 16],
                                 NCOL, NCOL, EMB, transpose=True)
            return t

        def body(rep):
            bT_q = {}
            for i in range(min(PREF, nblk)):
                bT_q[i] = issue_bT(i)

            # ---- prologue: A1^T = (a @ W1_a)^T as [h%128, h//128, m] in
            # one PSUM tile, then replicated x4 along columns with the b1
            # bias folded into each replication add.
            ps = plps.tile([128, 8, B_LOC], f32, tag="a1p")
            for ch in range(8):
                for ce in range(8):
                    nc.tensor.matmul(ps[:, ch, :],
                                     w1a[:, ce, ch * 128:(ch + 1) * 128],
                                     aT[:, ce, :], start=(ce == 0),
                                     stop=(ce == 7))
            for j in range(4):
                nc.vector.tensor_add(a1Tr[:, :, j * 128:(j + 1) * 128],
                                     ps[:], b1x[:])

            # ---- software-pipelined main loop:
            #   iter N: prep block N (DVE/ACT/DMA), matmuls block N-1,
            #           layer2 + scores block N-2.
            st = {}      # block -> (abT, bs) moving tensors
            hT_s = {}    # block -> [hT per ch]
            for it in range(nblk + 2):
                # stage B: matmuls + a1 add + lrelu for block `it-1`
                bm = it - 1
                if 0 <= bm < nblk and bm in st:
                    nj, NCOL, i0 = block_geom(bm)
                    abT, bs = st.pop(bm)
                    hts = []
                    for ch in range(8):
                        hp = (None if probe_ve else
                              pp.tile([128, NCOL], f32, tag="hp"))
                        if not probe_ve:
                            # fp8 DoubleRow: each pass contracts two
                            # 128-row planes (3-D APs [128, 2, cols])
                            for ce in range(4):
                                nc.tensor.matmul(
                                    hp[:],
                                    w1ab[:, 2 * ce:2 * ce + 2,
                                         ch * 128:(ch + 1) * 128],
                                    abT[:, 2 * ce:2 * ce + 2, 0:NCOL],
                                    start=(ce == 0), stop=False,
                                    perf_mode=DR)
                            for ce in range(5):
                                nc.tensor.matmul(
                                    hp[:],
                                    w1bx[:, 2 * ce:2 * ce + 2,
                                         ch * 128:(ch + 1) * 128],
                                    bs[:, 2 * ce:2 * ce + 2, 0:NCOL],
                                    start=False, stop=(ce == 4),
                                    perf_mode=DR)
                        # a-term added into PSUM on DVE (full bf16 A1)
                        if probe_ve:
                            nc.vector.tensor_add(dadd[:, 0:NCOL],
                                                 dadd[:, 0:NCOL],
                                                 a1Tr[:, ch, 0:NCOL])
                        elif not probe_pe:
                            nc.vector.tensor_add(hp[:], hp[:],
                                                 a1Tr[:, ch, 0:NCOL])
                        hT = hpool.tile([128, NCOL], bf16, tag="hT",
                                        name=f"hT{bm}_{ch}")
                        # leaky_relu(x) = max(x, 0.01*x)
                        if SIM_SAFE or DBG_NO_LRELU:  # CoreSim lacks Lrelu
                            lt = hpool.tile([128, NCOL], f32, tag="lt")
                            nc.vector.tensor_scalar_mul(lt[:], hp[:], 0.01)
                            nc.vector.tensor_max(hT[:], hp[:], lt[:])
                        elif probe_ve:
                            nc.scalar.activation(hT[:], aTr[:, 0:4, 0:NCOL],
                                                 AF.Lrelu, alpha=0.01)
                        else:
                            nc.scalar.activation(hT[:], hp[:], AF.Lrelu,
                                                 alpha=0.01)
                        hts.append(hT)
                    hT_s[bm] = hts

                # stage A: prep moving tensors for block `it`
                if it < nblk:
                    nj, NCOL, i0 = block_geom(it)
                    bT = bT_q.pop(it)
                    if it + PREF < nblk:
                        bT_q[it + PREF] = issue_bT(it + PREF)
                    if not DBG_GONLY:
                        if probe_pe:
                            st[it] = (w1ab, w1bx)  # const moving operands
                        else:
                            abT = wpool.tile([128, 8, NCOL], f8, tag="abT")
                            bs = bpool.tile([128, 10, NCOL], f8, tag="bs")
                            nc.sync.dma_start(
                                bs[:, 8:10, :],
                                px_d[:, :, it * 4:it * 4 + nj, :])
                            nc.vector.tensor_mul(abT[:], bT[:],
                                                 aTr[:, :, 0:NCOL])
                            # bT -> e4m3 cast split across ACT and DVE
                            nc.scalar.activation(bs[:, 0:4, :],
                                                 bT[:, 0:4, :], AF.Copy)
                            nc.vector.tensor_copy(bs[:, 4:8, :],
                                                  bT[:, 4:8, :])
                            st[it] = (abT, bs)

                # stage C: merged layer2 + scores for block `it-2` (its
                # lrelus finished an iteration ago - no PE stall)
                bl = it - 2
                if 0 <= bl < nblk and bl in hT_s:
                    njl, NCOLl, _ = block_geom(bl)
                    c0 = bl * COLS_PER_BLK
                    if probe_ve:
                        ff = roughf[0:1, c0:c0 + NCOLl]
                    else:
                        ff = fp.tile([1, NCOLl], f32, tag="ff")
                        for ch in range(8):
                            nc.tensor.matmul(ff[:], wout[:, ch:ch + 1],
                                             hT_s[bl][ch][:],
                                             start=(ch == 0), stop=(ch == 7))
                    sc = spool.tile([1, NCOLl], f32, tag="sc")
                    # scores = ffnn + b_out + rough
                    nc.vector.scalar_tensor_tensor(
                        sc[:], ff, bout[0:1, 0:1],
                        roughf[0:1, c0:c0 + NCOLl],
                        op0=ALU.add, op1=ALU.add)
                    nc.sync.dma_start(out_d[0:1, c0:c0 + NCOLl], sc[:])
                    del hT_s[bl]

        for rep in range(max(DBG_REPEAT, 1)):
            body(rep)

    nc.compile()
    return nc


def _shard(inputs):
    am = np.asarray(inputs["all_mentions"], np.float32)
    mb_ = np.asarray(inputs["mentions_batch"], np.float32)
    pw = np.asarray(inputs["pw_batch"], np.float32)
    rough = np.asarray(inputs["top_rough_scores_batch"], np.float32)
    W1 = np.asarray(inputs["W1"], np.float32)
    b1 = np.asarray(inputs["b1"], np.float32)
    Wout = np.asarray(inputs["W_out"], np.float32)
    bout = np.asarray(inputs["b_out"], np.float32)
    idx = np.asarray(inputs["top_indices_batch"])

    am_bf = am.astype(_BF16)

    def wblock(rows, dt=_BF16):  # [1024, 1024] -> [128, 8, 1024] (e%128, e//128, h)
        return np.ascontiguousarray(
            rows.reshape(8, 128, HID).transpose(1, 0, 2)).astype(dt)

    w1a = wblock(W1[0:1024])
    w1ab = wblock(W1[2048:3072], _F8)
    # stationary 10-plane fp8 block: [W1_b e-chunks; W1_pw pad; 0]
    w1bx = np.zeros((128, 10, HID), _F8)
    w1bx[:, 0:8, :] = wblock(W1[1024:2048], _F8)
    w1bx[0:64, 8, :] = W1[3072:3136].astype(_F8)   # W1_pw rows (64)
    wout = np.ascontiguousarray(Wout[:, 0].reshape(8, 128).T).astype(_BF16)
    b1x = np.ascontiguousarray(np.broadcast_to(
        b1.reshape(8, 128).T[:, :, None], (128, 8, B_LOC))).astype(_BF16)
    boutr = np.full((128, 1), bout[0], np.float32)

    def wrap_idx(flat, pad_cols):
        # [16, n/16] wrapped block, replicated across all 8 GPSIMD-core
        # partition groups (the odd Q7 core reads partitions 16-31).
        return np.tile(flat.reshape(pad_cols, 16).T, (8, 1))

    in_maps = []
    for c in range(CORES):
        sl = slice(c * B_LOC, (c + 1) * B_LOC)
        aT = np.ascontiguousarray(
            mb_[sl].T.reshape(8, 128, B_LOC).transpose(1, 0, 2)).astype(_BF16)
        aTr = np.ascontiguousarray(np.tile(aT, (1, 1, 4)))
        px = np.zeros((128, 2, N_ANTS, B_LOC), np.float32)
        px[0:64, 0, :, :] = pw[sl].transpose(2, 1, 0)   # pwT [f, j, m]
        idx_flat = np.ascontiguousarray(idx[sl].astype(np.int16).T).reshape(G)
        in_maps.append({
            "am": am_bf,
            "aT": aT,
            "aTr": aTr,
            "px": px.astype(_F8),
            "roughf": np.ascontiguousarray(
                rough[sl].T.reshape(1, G)),             # flat g = j*128+m
            "w1a": w1a, "w1bx": w1bx, "w1ab": w1ab,
            "wout": wout, "b1x": b1x, "boutr": boutr,
            "idx": wrap_idx(idx_flat, G // 16),
        })
    return in_maps


def _assemble(outs):
    """Per-core flat [1, 6400] score rows -> full [1024, 51] output."""
    full = np.empty((BATCH, 1 + N_ANTS), np.float32)
    full[:, 0] = EPS
    for c, o in enumerate(outs):
        full[c * B_LOC:(c + 1) * B_LOC, 1:] = \
            np.asarray(o).reshape(N_ANTS, B_LOC).T
    return full


def get_nc():
    if "nc" not in _cache:
        _cache["nc"] = _build_nc()
    return _cache["nc"]


def kernel(**inputs):
    from concourse.bass_utils import run_bass_kernel_spmd

    nc = get_nc()
    in_maps = _shard(inputs)
    res = run_bass_kernel_spmd(nc, in_maps, core_ids=list(range(CORES)))
    return _assemble([r["out"] for r in res.results])
